# revision 11
# baseline (speedup 1.0000x reference)
"""CGC (Customized Gate Control) MoE kernel for Trainium2, 8 NeuronCores.

Problem: 3 inputs x_{shared,task1,task2} [4096, 1024]; three expert groups
(sh/t1/t2) of 4 experts each; expert = fc2(relu(fc1(x))) with
fc1: 1024->2048, fc2: 2048->512; three softmax gates; outputs
(out_sh, out1, out2) each [4096, 512] as gate-weighted sums of expert
outputs.

Sharding: data-parallel over batch across 8 cores (512 rows/core), all
weights replicated. No collectives.

All matmuls run in bf16 (rel err ~3.8e-3 on the real data, well under the
2e-2 gate; same 1 cycle/row PE rate as fp32r but half the weight DMA and
SBUF traffic). fp8-DoubleRow schemes were measured in situ and are slower
(HW runs DR at ~1 cyc/out-row, so the accuracy-required 1.5 DR instrs per
K-tile lose to fp32r/bf16's 1.0).

Host-side prep in kernel(): weights cast to bf16; x transposed to [I, B]
and cast to bf16, so no on-chip transposes are needed (saves ~25k PE
cycles + DVE copies).

Per-core dataflow (batch tile b=512, partition tiles of 128):
  - xT [128, IT, 512] bf16 DMA'd directly (host pre-transposed)
  - gates: logits = xT.T @ wg + bg (PE) -> softmax (DVE+ACT), batch-major
  - per expert e: hT[ht] = relu(W1[:,ht].T @ xT + b1) (PE + DVE/ACT), bf16
                  o[bt] += hT[:,bt].T @ W2[ht] over ht (PE, PSUM accum)
                  o[bt] += ones.T @ b2 (PE)
                  acc[head][bt] (+)= g[head][:,e] * o[bt] (DVE)
  - store acc -> outputs.
"""
import sys
from contextlib import nullcontext

if "/opt/trn_rl_repo" not in sys.path:
    sys.path.insert(0, "/opt/trn_rl_repo")

import numpy as np

import concourse.bass as bass
import concourse.mybir as mybir
from concourse import bacc
from concourse.tile import TileContext
from concourse.masks import make_identity

B, I, H, O = 4096, 1024, 2048, 512
E = 4                      # experts per group
N_CORES = 8
BL = B // N_CORES          # 512 rows per core
BT = BL // 128             # 4 batch tiles
IT = I // 128              # 8 input tiles
HT = H // 128              # 16 hidden tiles

F32 = mybir.dt.float32
BF16 = mybir.dt.bfloat16

GROUPS = ("t1", "t2", "sh")
GATE_W = {"sh": 2 * E + E, "t1": E + E, "t2": E + E}  # 12, 8, 8


# (group, e) -> list of (head, gate_name, gate_col)
def _contribs(grp, e):
    if grp == "t1":
        return [("o1", "t1", e), ("osh", "sh", e)]
    if grp == "t2":
        return [("o2", "t2", e), ("osh", "sh", E + e)]
    return [("o1", "t1", E + e), ("o2", "t2", E + e), ("osh", "sh", 2 * E + e)]


def build_nc(loop_reps=None, mode="full"):
    """Build the per-core kernel. loop_reps wraps the whole body in a
    hardware For_i loop (used by the timing harness)."""
    nc = bacc.Bacc(None)

    # ---- DRAM parameters ----------------------------------------------
    # xT_{g}: host-transposed [I, BL] bf16
    xs = {g: nc.declare_dram_parameter(f"xT_{g}", [I, BL], BF16, isOutput=False)
          for g in GROUPS}
    w1 = {g: nc.declare_dram_parameter(f"w1_{g}", [E, I, H], BF16, isOutput=False)
          for g in GROUPS}
    b1 = {g: nc.declare_dram_parameter(f"b1_{g}", [E, H], F32, isOutput=False)
          for g in GROUPS}
    w2 = {g: nc.declare_dram_parameter(f"w2_{g}", [E, H, O], BF16, isOutput=False)
          for g in GROUPS}
    b2 = {g: nc.declare_dram_parameter(f"b2_{g}", [E, O], BF16, isOutput=False)
          for g in GROUPS}
    wg = {g: nc.declare_dram_parameter(f"wg_{g}", [I, GATE_W[g]], BF16, isOutput=False)
          for g in GROUPS}
    bg = {g: nc.declare_dram_parameter(f"bg_{g}", [GATE_W[g]], BF16, isOutput=False)
          for g in GROUPS}
    outs = {h: nc.declare_dram_parameter(h, [BL, O], F32, isOutput=True)
            for h in ("osh", "o1", "o2")}

    with TileContext(nc) as tc:
        with tc.tile_pool(name="persist", bufs=1) as pp, \
             tc.tile_pool(name="work", bufs=1) as pw, \
             tc.tile_pool(name="ps", bufs=1, space="PSUM") as ps:
            # persistent SBUF: xT per group, gates, accumulators, consts
            xT = {g: pp.tile([128, IT, BL], BF16, name=f"xT_{g}") for g in GROUPS}
            gsb = {g: pp.tile([128, BT, GATE_W[g]], F32, name=f"g_{g}")
                   for g in GROUPS}
            acc = {h: pp.tile([128, BT, O], F32, name=f"acc_{h}")
                   for h in ("osh", "o1", "o2")}
            ones_f = pp.tile([1, 128], F32, name="ones_f")
            nc.gpsimd.memset(ones_f[:, :], 1.0)
            ones = pp.tile([1, 128], BF16, name="ones")
            nc.vector.tensor_copy(ones[:, :], ones_f[:, :])
            ident = pp.tile([128, 128], F32, name="ident")
            make_identity(nc, ident[:, :])
            # warm the ACT function table before the loop so the in-loop
            # activations (Copy/Exp/Relu share one table set) never pay the
            # ~1.3us InstLoadActFuncSet inside an iteration
            warm = pp.tile([1, 128], F32, name="warm")
            nc.scalar.activation(warm[:, :], ones_f[:, :],
                                 mybir.ActivationFunctionType.Exp,
                                 bias=0.0, scale=1.0)
            # per-(head, bt) gate-weighted fc2-bias mixes, seeded into the
            # first gated accumulation of each head
            b2m = {(h, bt): pp.tile([128, O], F32, name=f"b2m_{h}_{bt}")
                   for h in ("osh", "o1", "o2") for bt in range(BT)}

            gate_w = {}
            loop_cm = tc.For_i(0, loop_reps, 1) if loop_reps else nullcontext()
            with loop_cm:
                # ---- Phase A: x loads + gates --------------------------
                for g in GROUPS:
                    nc.sync.dma_start(
                        xT[g][:, :, :],
                        xs[g].rearrange("(it p) b -> p it b", p=128))
                    wg_sb = pw.tile([128, IT, GATE_W[g]], BF16, tag=f"wg{g}",
                                    bufs=1, name=f"wg_{g}_sb")
                    nc.sync.dma_start(
                        wg_sb[:, :, :],
                        wg[g].rearrange("(it p) e -> p it e", p=128))
                    bg_sb = pw.tile([1, GATE_W[g]], BF16, tag=f"bg{g}", bufs=1,
                                    name=f"bg_{g}_sb")
                    nc.sync.dma_start(bg_sb[:, :], bg[g][None, :])
                    gate_w[g] = (wg_sb, bg_sb)
                gate_idx = 0
                for g in GROUPS:
                    wg_sb, bg_sb = gate_w[g]
                    for bt in range(BT):
                        # rotate gate logits over all 8 PSUM banks (the ph
                        # pool's 4 + the po banks; the expert phase hasn't
                        # started yet) so gate matmuls never WAR-stall
                        if gate_idx % 2 == 0:
                            gps = ps.tile([128, GATE_W[g]], F32, tag="ph",
                                          bufs=4, name=f"gps_{g}_{bt}")
                        else:
                            gtag = ("poA0", "poA1", "poB0", "poB1")[
                                (gate_idx // 2) % 4]
                            gps = ps.tile([128, GATE_W[g]], F32, tag=gtag,
                                          bufs=1, name=f"gps_{g}_{bt}")
                        for it in range(IT):
                            nc.tensor.matmul(
                                gps[:, :],
                                xT[g][:, it, bt * 128:(bt + 1) * 128],
                                wg_sb[:, it, :],
                                start=(it == 0), stop=False)
                        nc.tensor.matmul(gps[:, :], ones[:, :], bg_sb[:, :],
                                         start=False, stop=True)
                        # drain logits PSUM->SBUF immediately (frees the PSUM
                        # bank for the next gate matmul; the softmax chain
                        # below reads SBUF only); alternate ACT/DVE
                        glog = pw.tile([128, GATE_W[g]], F32, tag="glog",
                                       bufs=4, name=f"glog_{g}_{bt}")
                        if gate_idx % 2 == 0:
                            nc.scalar.copy(glog[:, :], gps[:, :])
                        else:
                            nc.vector.tensor_copy(glog[:, :], gps[:, :])
                        gate_idx += 1
                        # softmax over free dim
                        mx = pw.tile([128, 1], F32, tag="mx", bufs=2,
                                     name=f"mx_{g}_{bt}")
                        nc.vector.reduce_max(mx[:, :], glog[:, :],
                                             axis=mybir.AxisListType.X)
                        nmx = pw.tile([128, 1], F32, tag="nmx", bufs=2,
                                      name=f"nmx_{g}_{bt}")
                        nc.vector.tensor_scalar_mul(nmx[:, :], mx[:, :], -1.0)
                        ex = pw.tile([128, GATE_W[g]], F32, tag="ex", bufs=2,
                                     name=f"ex_{g}_{bt}")
                        nc.scalar.activation(ex[:, :], glog[:, :],
                                             mybir.ActivationFunctionType.Exp,
                                             bias=nmx[:, :], scale=1.0)
                        sm = pw.tile([128, 1], F32, tag="sm", bufs=2,
                                     name=f"sm_{g}_{bt}")
                        nc.vector.reduce_sum(sm[:, :], ex[:, :],
                                             axis=mybir.AxisListType.X)
                        rs = pw.tile([128, 1], F32, tag="rs", bufs=2,
                                     name=f"rs_{g}_{bt}")
                        nc.vector.reciprocal(rs[:, :], sm[:, :])
                        nc.vector.tensor_scalar_mul(gsb[g][:, bt, :], ex[:, :],
                                                    rs[:, :])

                # ---- fc2-bias head mixes: b2m[head] = gates @ b2cat ------
                # b2cat rows follow the gate column order of each head
                HEAD_CAT = {"o1": ("t1", ("t1", "sh")),
                            "o2": ("t2", ("t2", "sh")),
                            "osh": ("sh", ("t1", "t2", "sh"))}
                for h, (gg, srcs) in HEAD_CAT.items():
                    gw = GATE_W[gg]
                    cat = pw.tile([gw, O], BF16, tag=f"b2c_{h}", bufs=1,
                                  name=f"b2cat_{h}")
                    row = 0
                    for s in srcs:
                        nc.sync.dma_start(cat[row:row + E, :], b2[s][:, :])
                        row += E
                    for bt in range(BT):
                        gt_ps = ps.tile([gw, 128], F32, tag="ph", bufs=4,
                                        name=f"gtp_{h}_{bt}")
                        nc.tensor.transpose(gt_ps[:, :], gsb[gg][:, bt, :],
                                            ident[:, :])
                        gt_sb = pw.tile([gw, 128], BF16, tag="gt", bufs=4,
                                        name=f"gt_{h}_{bt}")
                        nc.vector.tensor_copy(gt_sb[:, :], gt_ps[:, :])
                        bm_ps = ps.tile([128, O], F32, tag="ph", bufs=4,
                                        name=f"bmp_{h}_{bt}")
                        nc.tensor.matmul(bm_ps[:, :], gt_sb[:, :], cat[:, :],
                                         start=True, stop=True)
                        nc.scalar.copy(b2m[(h, bt)][:, :], bm_ps[:, :])

                # ---- Phase B: experts ----------------------------------
                # fc2 is split into two bt-pair passes: pass A (bt 0,1) is
                # software-pipelined against fc1 (emitted SKEW fc1-blocks
                # late so the relu latency hides); pass B (bt 2,3) runs as a
                # block at the expert tail, emitted BEFORE fc2A(14),(15) so
                # its ~8us of PE work covers the last relus' latency. Each po
                # PSUM pair is then reused only after a multi-us window, so
                # the ACT/DVE drains are never on PE's critical path. The
                # gated accumulation (TensorScalarPtr) is DVE-only on trn2
                # (Pool rejects it). The fc2 bias arrives via the b2m seed
                # of the first accumulate.
                first_seen = set()
                HTG = 512 // 128  # ht-tiles per W1/W2 column block

                def drain_and_accum(g, e, bts, psum_o):
                    for i, bt in enumerate(bts):
                        o_sb = pw.tile([128, O], F32, tag="o_sb", bufs=4,
                                       name=f"osb_{g}{e}_{bt}")
                        if i == 0:
                            nc.scalar.copy(o_sb[:, :], psum_o[bt][:, :])
                        else:
                            nc.vector.tensor_copy(o_sb[:, :], psum_o[bt][:, :])
                        for head, gate, col in _contribs(g, e):
                            gcol = gsb[gate][:, bt, col:col + 1]
                            if (head, bt) not in first_seen:
                                src = b2m[(head, bt)]
                                first_seen.add((head, bt))
                            else:
                                src = acc[head]
                            nc.vector.scalar_tensor_tensor(
                                acc[head][:, bt, :], o_sb[:, :],
                                gcol,
                                src[:, bt, :] if src is acc[head]
                                else src[:, :],
                                op0=mybir.AluOpType.mult,
                                op1=mybir.AluOpType.add)

                SKEW = 2
                step = 0
                for g in GROUPS:
                    for e in range(E):
                        b1_sb = pw.tile([128, HT], F32, tag="b1", bufs=2,
                                        name=f"b1_{g}{e}")
                        nc.sync.dma_start(
                            b1_sb[:, :],
                            b1[g][e].rearrange("(ht p) -> p ht", p=128))
                        poA = [ps.tile([128, O], F32, tag=f"poA{bt}", bufs=1,
                                       name=f"poA_{g}{e}_{bt}")
                               for bt in range(2)]
                        poB = [ps.tile([128, O], F32, tag=f"poB{bt}", bufs=1,
                                       name=f"poB_{g}{e}_{bt}")
                               for bt in range(2)]
                        hts = {}
                        w2ts = {}

                        def emit_fc2A(ht):
                            hT_, w2t_, ht4_ = hts[ht]
                            for bt in range(2):
                                nc.tensor.matmul(
                                    poA[bt][:, :],
                                    hT_[:, bt * 128:(bt + 1) * 128],
                                    w2t_[:, ht4_, :],
                                    start=(ht == 0), stop=(ht == HT - 1))

                        for ht in range(HT):
                            htg, ht4 = divmod(ht, HTG)
                            if ht4 == 0:
                                # W1 column block [1024, 512] -> 1KB DMA beats
                                w1t = pw.tile([128, IT, 512], BF16, tag="w1",
                                              bufs=3, name=f"w1_{g}{e}_{htg}")
                                nc.sync.dma_start(
                                    w1t[:, :, :],
                                    w1[g][e, :, htg * 512:(htg + 1) * 512]
                                    .rearrange("(it p) h -> p it h", p=128))
                                # W2 row block [512, 512] -> 1KB DMA beats;
                                # bufs=6: all 4 blocks of an expert stay live
                                # until pass B reads them again
                                w2t = pw.tile([128, HTG, O], BF16, tag="w2",
                                              bufs=6, name=f"w2_{g}{e}_{htg}")
                                nc.sync.dma_start(
                                    w2t[:, :, :],
                                    w2[g][e, htg * 512:(htg + 1) * 512, :]
                                    .rearrange("(hh p) o -> p hh o", p=128))

                            ph = ps.tile([128, BL], F32, tag="ph", bufs=4,
                                         name=f"ph_{g}{e}_{ht}")
                            for it in range(IT):
                                nc.tensor.matmul(
                                    ph[:, :],
                                    w1t[:, it, ht4 * 128:(ht4 + 1) * 128],
                                    xT[g][:, it, :],
                                    start=(it == 0),
                                    stop=(it == IT - 1))
                            # hT bufs=18: all 16 ht tiles of an expert stay
                            # live until fc2 pass B consumes them
                            hT = pw.tile([128, BL], BF16, tag="hT", bufs=18,
                                         name=f"hT_{g}{e}_{ht}")
                            # relu(ph + b1) -> bf16; alternate DVE/ACT to
                            # split the epilogue load across both engines
                            if step % 2 == 0:
                                nc.vector.tensor_scalar(
                                    hT[:, :], ph[:, :],
                                    b1_sb[:, ht:ht + 1], 0.0,
                                    op0=mybir.AluOpType.add,
                                    op1=mybir.AluOpType.max)
                            else:
                                nc.scalar.activation(
                                    hT[:, :], ph[:, :],
                                    mybir.ActivationFunctionType.Relu,
                                    bias=b1_sb[:, ht:ht + 1], scale=1.0)
                            hts[ht] = (hT, w2t, ht4)
                            if ht >= SKEW:
                                emit_fc2A(ht - SKEW)
                            step += 1
                        # fc2 pass B first: it depends only on old hTs, so
                        # its ~8us of PE work covers the latency of the last
                        # relus before fc2A(14),(15) need them
                        for ht in range(HT):
                            hT_, w2t_, ht4_ = hts[ht]
                            for bt in (2, 3):
                                nc.tensor.matmul(
                                    poB[bt - 2][:, :],
                                    hT_[:, bt * 128:(bt + 1) * 128],
                                    w2t_[:, ht4_, :],
                                    start=(ht == 0), stop=(ht == HT - 1))
                        for ht in range(HT - SKEW, HT):
                            emit_fc2A(ht)
                        drain_and_accum(g, e, (2, 3), {2: poB[0], 3: poB[1]})
                        drain_and_accum(g, e, (0, 1), {0: poA[0], 1: poA[1]})

                # ---- store outputs -----------------------------------
                for h in ("osh", "o1", "o2"):
                    for bt in range(BT):
                        nc.sync.dma_start(outs[h][bt * 128:(bt + 1) * 128, :],
                                          acc[h][:, bt, :])

    nc.finalize()
    return nc


_NC_CACHE = None


def _get_nc():
    global _NC_CACHE
    if _NC_CACHE is None:
        _NC_CACHE = build_nc()
    return _NC_CACHE


def host_prep(inputs):
    """Cast weights to bf16 and transpose x inputs; returns dict of full
    (unsharded) arrays keyed by DRAM parameter name (x keyed per group
    with the full [I, B] transpose; caller slices columns per core)."""
    import ml_dtypes
    bf16 = ml_dtypes.bfloat16
    np_in = {k: np.asarray(v) for k, v in inputs.items()}
    prep = {}
    for g, src in (("sh", "x_shared"), ("t1", "x_task1"), ("t2", "x_task2")):
        prep[f"xT_{g}"] = np.ascontiguousarray(
            np_in[src].astype(np.float32).T.astype(bf16))  # [I, B]
    for g in GROUPS:
        for pfx in ("w1", "w2", "wg", "b2", "bg"):
            prep[f"{pfx}_{g}"] = np.ascontiguousarray(
                np_in[f"{pfx}_{g}"].astype(np.float32).astype(bf16))
        prep[f"b1_{g}"] = np.ascontiguousarray(
            np_in[f"b1_{g}"].astype(np.float32))
    return prep


def kernel(**inputs) -> tuple:
    from concourse.bass_utils import run_bass_kernel_spmd

    nc = _get_nc()
    prep = host_prep(inputs)
    in_maps = []
    for c in range(N_CORES):
        sl = slice(c * BL, (c + 1) * BL)
        m = {f"xT_{g}": np.ascontiguousarray(prep[f"xT_{g}"][:, sl])
             for g in GROUPS}
        for g in GROUPS:
            for pfx in ("w1", "b1", "w2", "b2", "wg", "bg"):
                m[f"{pfx}_{g}"] = prep[f"{pfx}_{g}"]
        in_maps.append(m)

    # rare transient NRT_EXEC_UNIT_UNRECOVERABLE crashes have been observed
    # on this fabric; retry a couple of times before giving up
    last_err = None
    for attempt in range(3):
        try:
            r = run_bass_kernel_spmd(nc, in_maps, list(range(N_CORES)))
            break
        except Exception as ex:  # noqa: BLE001
            last_err = ex
            import time as _time
            _time.sleep(5 * (attempt + 1))
    else:
        raise last_err
    out_sh = np.concatenate([r.results[c]["osh"] for c in range(N_CORES)], axis=0)
    out1 = np.concatenate([r.results[c]["o1"] for c in range(N_CORES)], axis=0)
    out2 = np.concatenate([r.results[c]["o2"] for c in range(N_CORES)], axis=0)
    return (out_sh, out1, out2)



# revision 17
# speedup vs baseline: 1.0073x; 1.0073x over previous
"""CGC (Customized Gate Control) MoE kernel for Trainium2, 8 NeuronCores.

Problem: 3 inputs x_{shared,task1,task2} [4096, 1024]; three expert groups
(sh/t1/t2) of 4 experts each; expert = fc2(relu(fc1(x))) with
fc1: 1024->2048, fc2: 2048->512; three softmax gates; outputs
(out_sh, out1, out2) each [4096, 512] as gate-weighted sums of expert
outputs.

Sharding: data-parallel over batch across 8 cores (512 rows/core), all
weights replicated. No collectives.

All matmuls run in bf16 (rel err ~3.8e-3 on the real data, well under the
2e-2 gate; same 1 cycle/row PE rate as fp32r but half the weight DMA and
SBUF traffic).

Precision study (measured on the real data + HW microbenches):
  - fp8e4 DoubleRow really is 2x bf16 per instruction on HW (0.60 vs 1.21
    cyc/out-row back-to-back), but pure-e4m3 accuracy is 4.4e-2 per fp8
    layer (>2e-2 gate; per-column scales don't help), and the accurate
    hi/lo 3-term scheme needs 1.5 DR instrs per K-tile -> 1.5x slower
    than bf16. No fp8 scheme both passes accuracy and beats bf16.
  - HW microbench: back-to-back full-width bf16 matmuls run at ~1.21
    cyc/row (power-throttled to ~2.0 GHz); 64-wide stationary runs at
    ~1.02 (full 2.4 GHz at half array power); extra instructions cost
    ~2.2 ns each (HW decode); matmul weight loads DO overlap. The kernel
    at ~95% PE duty is essentially at the power/thermal wall, so stall
    removal yields little on HW (session-to-session thermal drift is
    +-7%, larger than remaining stall savings).
  - PE floor: 12 experts x (fc1 65536 + fc2 32768) = 1.18M cycles
    = 491 us @2.4GHz, ~590 us at the hot throttled clock.

Host-side prep in kernel(): weights cast to bf16; x transposed to [I, B]
and cast to bf16, so no on-chip transposes are needed (saves ~25k PE
cycles + DVE copies).

Per-core dataflow (batch tile b=512, partition tiles of 128):
  - xT [128, IT, 512] bf16 DMA'd directly (host pre-transposed)
  - gates: logits = xT.T @ wg + bg (PE) -> drain to SBUF -> softmax
    (DVE+ACT) batch-major; logits PSUM rotates over all 8 banks
  - b2m gate-weighted fc2-bias mixes ride under the first expert's fc1
  - per expert e: hT[ht] = relu(W1[:,ht].T @ xT + b1) (PE + DVE/ACT), bf16
                  fc2 in two bt-pair passes (A: bt 0,1 pipelined vs fc1;
                  B: bt 2,3 en-bloc at the tail, before fc2A(14,15) so the
                  last relus' latency hides under it); PSUM drains split
                  ACT/DVE with multi-us WAR windows
                  acc[head][bt] (+)= g[head][:,e] * o[bt] (DVE stt;
                  TensorScalarPtr is DVE-only on trn2 - Pool rejects it)
  - store acc -> outputs.
"""
import sys
from contextlib import nullcontext

if "/opt/trn_rl_repo" not in sys.path:
    sys.path.insert(0, "/opt/trn_rl_repo")

import numpy as np

import concourse.bass as bass
import concourse.mybir as mybir
from concourse import bacc
from concourse.tile import TileContext
from concourse.masks import make_identity

B, I, H, O = 4096, 1024, 2048, 512
E = 4                      # experts per group
N_CORES = 8
BL = B // N_CORES          # 512 rows per core
BT = BL // 128             # 4 batch tiles
IT = I // 128              # 8 input tiles
HT = H // 128              # 16 hidden tiles

F32 = mybir.dt.float32
BF16 = mybir.dt.bfloat16

GROUPS = ("t1", "t2", "sh")
GATE_W = {"sh": 2 * E + E, "t1": E + E, "t2": E + E}  # 12, 8, 8


# (group, e) -> list of (head, gate_name, gate_col)
def _contribs(grp, e):
    if grp == "t1":
        return [("o1", "t1", e), ("osh", "sh", e)]
    if grp == "t2":
        return [("o2", "t2", e), ("osh", "sh", E + e)]
    return [("o1", "t1", E + e), ("o2", "t2", E + e), ("osh", "sh", 2 * E + e)]


def build_nc(loop_reps=None, mode="full"):
    """Build the per-core kernel. loop_reps wraps the whole body in a
    hardware For_i loop (used by the timing harness)."""
    nc = bacc.Bacc(None)

    # ---- DRAM parameters ----------------------------------------------
    # xT_{g}: host-transposed [I, BL] bf16
    xs = {g: nc.declare_dram_parameter(f"xT_{g}", [I, BL], BF16, isOutput=False)
          for g in GROUPS}
    w1 = {g: nc.declare_dram_parameter(f"w1_{g}", [E, I, H], BF16, isOutput=False)
          for g in GROUPS}
    b1 = {g: nc.declare_dram_parameter(f"b1_{g}", [E, H], F32, isOutput=False)
          for g in GROUPS}
    w2 = {g: nc.declare_dram_parameter(f"w2_{g}", [E, H, O], BF16, isOutput=False)
          for g in GROUPS}
    b2 = {g: nc.declare_dram_parameter(f"b2_{g}", [E, O], BF16, isOutput=False)
          for g in GROUPS}
    wg = {g: nc.declare_dram_parameter(f"wg_{g}", [I, GATE_W[g]], BF16, isOutput=False)
          for g in GROUPS}
    bg = {g: nc.declare_dram_parameter(f"bg_{g}", [GATE_W[g]], BF16, isOutput=False)
          for g in GROUPS}
    outs = {h: nc.declare_dram_parameter(h, [BL, O], F32, isOutput=True)
            for h in ("osh", "o1", "o2")}

    with TileContext(nc) as tc:
        with tc.tile_pool(name="persist", bufs=1) as pp, \
             tc.tile_pool(name="work", bufs=1) as pw, \
             tc.tile_pool(name="ps", bufs=1, space="PSUM") as ps:
            # persistent SBUF: xT per group, gates, accumulators, consts
            xT = {g: pp.tile([128, IT, BL], BF16, name=f"xT_{g}") for g in GROUPS}
            gsb = {g: pp.tile([128, BT, GATE_W[g]], F32, name=f"g_{g}")
                   for g in GROUPS}
            acc = {h: pp.tile([128, BT, O], F32, name=f"acc_{h}")
                   for h in ("osh", "o1", "o2")}
            ones_f = pp.tile([1, 128], F32, name="ones_f")
            nc.gpsimd.memset(ones_f[:, :], 1.0)
            ones = pp.tile([1, 128], BF16, name="ones")
            nc.vector.tensor_copy(ones[:, :], ones_f[:, :])
            ident = pp.tile([128, 128], F32, name="ident")
            make_identity(nc, ident[:, :])
            # warm the ACT function table before the loop so the in-loop
            # activations (Copy/Exp/Relu share one table set) never pay the
            # ~1.3us InstLoadActFuncSet inside an iteration
            warm = pp.tile([1, 128], F32, name="warm")
            nc.scalar.activation(warm[:, :], ones_f[:, :],
                                 mybir.ActivationFunctionType.Exp,
                                 bias=0.0, scale=1.0)
            # per-(head, bt) gate-weighted fc2-bias mixes, seeded into the
            # first gated accumulation of each head
            b2m = {(h, bt): pp.tile([128, O], F32, name=f"b2m_{h}_{bt}")
                   for h in ("osh", "o1", "o2") for bt in range(BT)}

            gate_w = {}
            loop_cm = tc.For_i(0, loop_reps, 1) if loop_reps else nullcontext()
            with loop_cm:
                # ---- Phase A: x loads + gates --------------------------
                for g in GROUPS:
                    nc.sync.dma_start(
                        xT[g][:, :, :],
                        xs[g].rearrange("(it p) b -> p it b", p=128))
                    wg_sb = pw.tile([128, IT, GATE_W[g]], BF16, tag=f"wg{g}",
                                    bufs=1, name=f"wg_{g}_sb")
                    nc.sync.dma_start(
                        wg_sb[:, :, :],
                        wg[g].rearrange("(it p) e -> p it e", p=128))
                    bg_sb = pw.tile([1, GATE_W[g]], BF16, tag=f"bg{g}", bufs=1,
                                    name=f"bg_{g}_sb")
                    nc.sync.dma_start(bg_sb[:, :], bg[g][None, :])
                    gate_w[g] = (wg_sb, bg_sb)
                gate_idx = 0
                for g in GROUPS:
                    wg_sb, bg_sb = gate_w[g]
                    for bt in range(BT):
                        # rotate gate logits over all 8 PSUM banks (the ph
                        # pool's 4 + the po banks; the expert phase hasn't
                        # started yet) so gate matmuls never WAR-stall
                        if gate_idx % 2 == 0:
                            gps = ps.tile([128, GATE_W[g]], F32, tag="ph",
                                          bufs=4, name=f"gps_{g}_{bt}")
                        else:
                            gtag = ("poA0", "poA1", "poB0", "poB1")[
                                (gate_idx // 2) % 4]
                            gps = ps.tile([128, GATE_W[g]], F32, tag=gtag,
                                          bufs=1, name=f"gps_{g}_{bt}")
                        for it in range(IT):
                            nc.tensor.matmul(
                                gps[:, :],
                                xT[g][:, it, bt * 128:(bt + 1) * 128],
                                wg_sb[:, it, :],
                                start=(it == 0), stop=False)
                        nc.tensor.matmul(gps[:, :], ones[:, :], bg_sb[:, :],
                                         start=False, stop=True)
                        # drain logits PSUM->SBUF immediately (frees the PSUM
                        # bank for the next gate matmul; the softmax chain
                        # below reads SBUF only); alternate ACT/DVE
                        glog = pw.tile([128, GATE_W[g]], F32, tag="glog",
                                       bufs=4, name=f"glog_{g}_{bt}")
                        if gate_idx % 2 == 0:
                            nc.scalar.copy(glog[:, :], gps[:, :])
                        else:
                            nc.vector.tensor_copy(glog[:, :], gps[:, :])
                        gate_idx += 1
                        # softmax over free dim
                        mx = pw.tile([128, 1], F32, tag="mx", bufs=2,
                                     name=f"mx_{g}_{bt}")
                        nc.vector.reduce_max(mx[:, :], glog[:, :],
                                             axis=mybir.AxisListType.X)
                        nmx = pw.tile([128, 1], F32, tag="nmx", bufs=2,
                                      name=f"nmx_{g}_{bt}")
                        nc.vector.tensor_scalar_mul(nmx[:, :], mx[:, :], -1.0)
                        ex = pw.tile([128, GATE_W[g]], F32, tag="ex", bufs=2,
                                     name=f"ex_{g}_{bt}")
                        nc.scalar.activation(ex[:, :], glog[:, :],
                                             mybir.ActivationFunctionType.Exp,
                                             bias=nmx[:, :], scale=1.0)
                        sm = pw.tile([128, 1], F32, tag="sm", bufs=2,
                                     name=f"sm_{g}_{bt}")
                        nc.vector.reduce_sum(sm[:, :], ex[:, :],
                                             axis=mybir.AxisListType.X)
                        rs = pw.tile([128, 1], F32, tag="rs", bufs=2,
                                     name=f"rs_{g}_{bt}")
                        nc.vector.reciprocal(rs[:, :], sm[:, :])
                        nc.vector.tensor_scalar_mul(gsb[g][:, bt, :], ex[:, :],
                                                    rs[:, :])

                # ---- fc2-bias head mixes: b2m[head] = gates @ b2cat ------
                # b2cat rows follow the gate column order of each head. The
                # cat DMAs are issued here; the 12 (head, bt) mix units are
                # NOT emitted now — they interleave under the first expert's
                # fc1 blocks (emit_b2m_unit below) so the softmax->transpose
                # ->matmul latency chain hides under fc1 compute.
                HEAD_CAT = {"o1": ("t1", ("t1", "sh")),
                            "o2": ("t2", ("t2", "sh")),
                            "osh": ("sh", ("t1", "t2", "sh"))}
                cats = {}
                for h, (gg, srcs) in HEAD_CAT.items():
                    gw = GATE_W[gg]
                    cat = pw.tile([gw, O], BF16, tag=f"b2c_{h}", bufs=1,
                                  name=f"b2cat_{h}")
                    row = 0
                    for s in srcs:
                        nc.sync.dma_start(cat[row:row + E, :], b2[s][:, :])
                        row += E
                    cats[h] = cat
                b2m_units = [(h, HEAD_CAT[h][0], bt)
                             for h in ("o1", "o2", "osh") for bt in range(BT)]

                def emit_b2m_unit(k):
                    h, gg, bt = b2m_units[k]
                    gw = GATE_W[gg]
                    # poB banks are idle during the first expert's fc1
                    gt_ps = ps.tile([gw, 128], F32, tag="poB0", bufs=1,
                                    name=f"gtp_{h}_{bt}")
                    nc.tensor.transpose(gt_ps[:, :], gsb[gg][:, bt, :],
                                        ident[:, :])
                    gt_sb = pw.tile([gw, 128], BF16, tag="gt", bufs=4,
                                    name=f"gt_{h}_{bt}")
                    nc.vector.tensor_copy(gt_sb[:, :], gt_ps[:, :])
                    bm_ps = ps.tile([128, O], F32, tag="poB1", bufs=1,
                                    name=f"bmp_{h}_{bt}")
                    nc.tensor.matmul(bm_ps[:, :], gt_sb[:, :], cats[h][:, :],
                                     start=True, stop=True)
                    nc.scalar.copy(b2m[(h, bt)][:, :], bm_ps[:, :])

                # ---- Phase B: experts ----------------------------------
                # fc2 is split into two bt-pair passes: pass A (bt 0,1) is
                # software-pipelined against fc1 (emitted SKEW fc1-blocks
                # late so the relu latency hides); pass B (bt 2,3) runs as a
                # block at the expert tail, emitted BEFORE fc2A(14),(15) so
                # its ~8us of PE work covers the last relus' latency. Each po
                # PSUM pair is then reused only after a multi-us window, so
                # the ACT/DVE drains are never on PE's critical path. The
                # gated accumulation (TensorScalarPtr) is DVE-only on trn2
                # (Pool rejects it). The fc2 bias arrives via the b2m seed
                # of the first accumulate.
                first_seen = set()
                HTG = 512 // 128  # ht-tiles per W1/W2 column block

                def drain_and_accum(g, e, bts, psum_o):
                    for i, bt in enumerate(bts):
                        o_sb = pw.tile([128, O], F32, tag="o_sb", bufs=4,
                                       name=f"osb_{g}{e}_{bt}")
                        if i == 0:
                            nc.scalar.copy(o_sb[:, :], psum_o[bt][:, :])
                        else:
                            nc.vector.tensor_copy(o_sb[:, :], psum_o[bt][:, :])
                        for head, gate, col in _contribs(g, e):
                            gcol = gsb[gate][:, bt, col:col + 1]
                            if (head, bt) not in first_seen:
                                src = b2m[(head, bt)]
                                first_seen.add((head, bt))
                            else:
                                src = acc[head]
                            nc.vector.scalar_tensor_tensor(
                                acc[head][:, bt, :], o_sb[:, :],
                                gcol,
                                src[:, bt, :] if src is acc[head]
                                else src[:, :],
                                op0=mybir.AluOpType.mult,
                                op1=mybir.AluOpType.add)

                SKEW = 2
                step = 0
                for g in GROUPS:
                    for e in range(E):
                        b1_sb = pw.tile([128, HT], F32, tag="b1", bufs=2,
                                        name=f"b1_{g}{e}")
                        nc.sync.dma_start(
                            b1_sb[:, :],
                            b1[g][e].rearrange("(ht p) -> p ht", p=128))
                        poA = [ps.tile([128, O], F32, tag=f"poA{bt}", bufs=1,
                                       name=f"poA_{g}{e}_{bt}")
                               for bt in range(2)]
                        poB = [ps.tile([128, O], F32, tag=f"poB{bt}", bufs=1,
                                       name=f"poB_{g}{e}_{bt}")
                               for bt in range(2)]
                        hts = {}
                        w2ts = {}

                        def emit_fc2A(ht):
                            hT_, w2t_, ht4_ = hts[ht]
                            for bt in range(2):
                                nc.tensor.matmul(
                                    poA[bt][:, :],
                                    hT_[:, bt * 128:(bt + 1) * 128],
                                    w2t_[:, ht4_, :],
                                    start=(ht == 0), stop=(ht == HT - 1))

                        for ht in range(HT):
                            htg, ht4 = divmod(ht, HTG)
                            if ht4 == 0:
                                # W1 column block [1024, 512] -> 1KB DMA beats
                                w1t = pw.tile([128, IT, 512], BF16, tag="w1",
                                              bufs=5, name=f"w1_{g}{e}_{htg}")
                                nc.sync.dma_start(
                                    w1t[:, :, :],
                                    w1[g][e, :, htg * 512:(htg + 1) * 512]
                                    .rearrange("(it p) h -> p it h", p=128))
                                # W2 row block [512, 512] -> 1KB DMA beats;
                                # bufs=6: all 4 blocks of an expert stay live
                                # until pass B reads them again
                                w2t = pw.tile([128, HTG, O], BF16, tag="w2",
                                              bufs=6, name=f"w2_{g}{e}_{htg}")
                                nc.sync.dma_start(
                                    w2t[:, :, :],
                                    w2[g][e, htg * 512:(htg + 1) * 512, :]
                                    .rearrange("(hh p) o -> p hh o", p=128))

                            ph = ps.tile([128, BL], F32, tag="ph", bufs=4,
                                         name=f"ph_{g}{e}_{ht}")
                            for it in range(IT):
                                nc.tensor.matmul(
                                    ph[:, :],
                                    w1t[:, it, ht4 * 128:(ht4 + 1) * 128],
                                    xT[g][:, it, :],
                                    start=(it == 0),
                                    stop=(it == IT - 1))
                            # hT bufs=18: all 16 ht tiles of an expert stay
                            # live until fc2 pass B consumes them
                            hT = pw.tile([128, BL], BF16, tag="hT", bufs=18,
                                         name=f"hT_{g}{e}_{ht}")
                            # relu(ph + b1) -> bf16; alternate DVE/ACT to
                            # split the epilogue load across both engines.
                            # ht 0,1 go to ACT: DVE still owes the previous
                            # expert's gated-accum stts at that point, and
                            # fc2A(0) needs relu(0) by fc1 block 2.
                            if ht >= 2 and step % 2 == 0:
                                nc.vector.tensor_scalar(
                                    hT[:, :], ph[:, :],
                                    b1_sb[:, ht:ht + 1], 0.0,
                                    op0=mybir.AluOpType.add,
                                    op1=mybir.AluOpType.max)
                            else:
                                nc.scalar.activation(
                                    hT[:, :], ph[:, :],
                                    mybir.ActivationFunctionType.Relu,
                                    bias=b1_sb[:, ht:ht + 1], scale=1.0)
                            hts[ht] = (hT, w2t, ht4)
                            if ht >= SKEW:
                                emit_fc2A(ht - SKEW)
                            # the 12 b2m mix units ride under the first
                            # expert's fc1 blocks (one per block, from ht=1
                            # so the t1 softmax has a block of slack)
                            if g == GROUPS[0] and e == 0 and \
                                    1 <= ht <= len(b2m_units):
                                emit_b2m_unit(ht - 1)
                            step += 1
                        # fc2 pass B first: it depends only on old hTs, so
                        # its ~8us of PE work covers the latency of the last
                        # relus before fc2A(14),(15) need them
                        for ht in range(HT):
                            hT_, w2t_, ht4_ = hts[ht]
                            for bt in (2, 3):
                                nc.tensor.matmul(
                                    poB[bt - 2][:, :],
                                    hT_[:, bt * 128:(bt + 1) * 128],
                                    w2t_[:, ht4_, :],
                                    start=(ht == 0), stop=(ht == HT - 1))
                        for ht in range(HT - SKEW, HT):
                            emit_fc2A(ht)
                        # drain B first: fc2B(15) retires ~2.5us before
                        # fc2A(15), so the B-side engine work overlaps the
                        # fc2A(14),(15) matmuls
                        drain_and_accum(g, e, (2, 3), {2: poB[0], 3: poB[1]})
                        drain_and_accum(g, e, (0, 1), {0: poA[0], 1: poA[1]})

                # ---- store outputs -----------------------------------
                for h in ("osh", "o1", "o2"):
                    for bt in range(BT):
                        nc.sync.dma_start(outs[h][bt * 128:(bt + 1) * 128, :],
                                          acc[h][:, bt, :])

    nc.finalize()
    return nc


_NC_CACHE = None


def _get_nc():
    global _NC_CACHE
    if _NC_CACHE is None:
        _NC_CACHE = build_nc()
    return _NC_CACHE


def host_prep(inputs):
    """Cast weights to bf16 and transpose x inputs; returns dict of full
    (unsharded) arrays keyed by DRAM parameter name (x keyed per group
    with the full [I, B] transpose; caller slices columns per core)."""
    import ml_dtypes
    bf16 = ml_dtypes.bfloat16
    np_in = {k: np.asarray(v) for k, v in inputs.items()}
    prep = {}
    for g, src in (("sh", "x_shared"), ("t1", "x_task1"), ("t2", "x_task2")):
        prep[f"xT_{g}"] = np.ascontiguousarray(
            np_in[src].astype(np.float32).T.astype(bf16))  # [I, B]
    for g in GROUPS:
        for pfx in ("w1", "w2", "wg", "b2", "bg"):
            prep[f"{pfx}_{g}"] = np.ascontiguousarray(
                np_in[f"{pfx}_{g}"].astype(np.float32).astype(bf16))
        prep[f"b1_{g}"] = np.ascontiguousarray(
            np_in[f"b1_{g}"].astype(np.float32))
    return prep


def kernel(**inputs) -> tuple:
    from concourse.bass_utils import run_bass_kernel_spmd

    nc = _get_nc()
    prep = host_prep(inputs)
    in_maps = []
    for c in range(N_CORES):
        sl = slice(c * BL, (c + 1) * BL)
        m = {f"xT_{g}": np.ascontiguousarray(prep[f"xT_{g}"][:, sl])
             for g in GROUPS}
        for g in GROUPS:
            for pfx in ("w1", "b1", "w2", "b2", "wg", "bg"):
                m[f"{pfx}_{g}"] = prep[f"{pfx}_{g}"]
        in_maps.append(m)

    # rare transient NRT_EXEC_UNIT_UNRECOVERABLE crashes have been observed
    # on this fabric; retry a couple of times before giving up
    last_err = None
    for attempt in range(3):
        try:
            r = run_bass_kernel_spmd(nc, in_maps, list(range(N_CORES)))
            break
        except Exception as ex:  # noqa: BLE001
            last_err = ex
            import time as _time
            _time.sleep(5 * (attempt + 1))
    else:
        raise last_err
    out_sh = np.concatenate([r.results[c]["osh"] for c in range(N_CORES)], axis=0)
    out1 = np.concatenate([r.results[c]["o1"] for c in range(N_CORES)], axis=0)
    out2 = np.concatenate([r.results[c]["o2"] for c in range(N_CORES)], axis=0)
    return (out_sh, out1, out2)

